# revision 3
# baseline (speedup 1.0000x reference)
"""Bass/Tile Trainium2 kernel for nn_AttentionSampling.

Problem: out = q + attention_downsampling(LN(q), LN(k), LN(v), factor=4)
  B=4, Sq=2048, Skv=8192, D=1024. Per query token s:
    w_f   = dot(LN(q)[s], LN(k)[4s+f])          f in 0..3  (no softmax)
    out[s] = q[s] + sum_f w_f * LN(v)[4s+f]

Key algebraic folding (valid for ln_weight==1, ln_bias==0, which is what
setup_inputs produces; a numpy fallback handles the general case):
    dot(LN(q), LN(k)) = aq*ak*(q.k - D*muq*muk)      a = rsqrt(var+eps)
    sum_f w_f*LN(v_f) = sum_f c_f*v_f - (sum_f c_f*muv_f)*ones,  c_f = w_f*av_f
so no normalized tensor is ever materialized: only raw dots + per-token stats.

v2: all-bf16 device I/O (inputs cast on host; rel_err ~5e-3 << 2e-2 gate),
halving HBM traffic 40MiB -> 20MiB per core, plus engine rebalancing:
  - DVE: k bn_stats, dots (STT w/ accum), v-sum reduce (per V_MODE)
  - ACT: q stats (2-pass accum) or v-sq, out = psum + bias(-d)
  - Pool: small [128,4] weight math + diag builds (no STT/reduce on Pool:
    this walrus rejects TensorScalarPtr and X-axis reduce on Pool)
  - PE : bf16 matmuls: psum = ident@q + sum_f diag(c_f)@v_f

Sharding: 8 cores = batch (4) x query-half (2). Each core owns 1024 windows:
q[1024,1024], k/v[1024,4,1024] (window-major view), out[1024,1024].
"""

import numpy as np


def _ensure_concourse():
    try:
        import concourse.bass  # noqa: F401
    except ImportError:
        import sys

        for p in ("/opt/trn_rl_repo", "/root/.axon_site/_ro/trn_rl_repo"):
            if p not in sys.path:
                sys.path.insert(0, p)


_ensure_concourse()

import concourse.bass as bass  # noqa: E402
import concourse.tile as tile  # noqa: E402
from concourse import mybir  # noqa: E402
from concourse.bass_utils import run_bass_kernel_spmd  # noqa: E402

# ---------------------------------------------------------------------------
# Walrus-compatibility shims.
#
# The walrus in this container rejects two things Tile's end-of-context tail
# emits: (a) the final Drain carrying >2 sem waits ("Too many sync wait
# commands"), and (b) EVENT_SEMAPHORE_RANGE_CLEAR ("ISA wrong length").
# Replace the tail with per-semaphore EventSemaphore instructions that wait
# for each sem's final value, then the normal all-engine barrier. A JSON-level
# pass additionally splits any instruction carrying more than MAX_WAITS sem
# waits into EventSemaphore wait carriers.
# ---------------------------------------------------------------------------

_MAX_WAITS = 1


def _patched_drain_and_barrier(self, tick_clock, wait_clock):
    nc = self.nc
    gc = tick_clock.global_clock
    sems = self.sems.allocated()  # proc idx -> SemaphoreHandle
    for proc in sorted(sems):
        h = sems[proc]
        if "DMA" not in h.name:
            continue  # engine sems are implied by stream completion
        final = int(gc[proc]) * 16
        if final > 0:
            nc.gpsimd.wait_ge(h, final)
    nc.all_engine_barrier()
    popped = nc._tile_sem_poison_stack.pop()
    assert popped is self._sem_poison


tile.TileContext._drain_and_barrier = _patched_drain_and_barrier

_orig_to_json_bytes = bass.Bass.to_json_bytes


def _to_json_bytes_compat(self):
    import orjson

    raw = _orig_to_json_bytes(self)
    d = orjson.loads(raw)
    changed = False
    for fn in d.get("functions", []):
        blocks = fn.get("basic_blocks") or fn.get("blocks") or []
        for bb in blocks:
            insts = bb.get("instructions", [])
            new_insts = []
            for inst in insts:
                waits = (inst.get("sync_info") or {}).get("on_wait") or []
                if len(waits) > _MAX_WAITS:
                    keep = waits[-_MAX_WAITS:]
                    excess = waits[:-_MAX_WAITS]
                    for i, wt in enumerate(excess):
                        new_insts.append(
                            {
                                "name": f"{inst['name']}_wsplit{i}",
                                "opcode": "EventSemaphore",
                                "engine": inst["engine"],
                                "ins": [],
                                "outs": [],
                                "debug": inst.get("debug"),
                                "sync_info": {"on_update": [], "on_wait": [wt]},
                            }
                        )
                    inst["sync_info"]["on_wait"] = keep
                    changed = True
                new_insts.append(inst)
            bb["instructions"] = new_insts
    return orjson.dumps(d) if changed else raw


bass.Bass.to_json_bytes = _to_json_bytes_compat

F32 = mybir.dt.float32
BF16 = mybir.dt.bfloat16
ALU = mybir.AluOpType
ACTF = mybir.ActivationFunctionType
AXL = mybir.AxisListType

B, SQ, SKV, D = 4, 2048, 8192, 1024
FACTOR = 4
N_CORES = 8
W_PER_CORE = B * SQ // N_CORES  # 1024 windows per core
P = 128  # windows per tile = SBUF partitions
LN_EPS = 1e-5
HALF = 512  # PSUM bank free-dim (f32)

# ---- engine-assignment tunables -------------------------------------------
STAT_DT = BF16  # bn_stats/bn_aggr stats dtype (bf16 enables 2-byte DVE modes)
Q_MODE = "act2"  # "bn" (DVE bn_stats) | "act2" (ACT Copy+Square accum)
V_MODE = "split"  # "bn" (DVE bn_stats) | "split" (ACT sq-accum + DVE sum-reduce)
K_MODE = "bn"  # "bn" only (DVE bn_stats)
OUT_MODE = "act"  # "act" (bias act + PE ident-q) | "dve" (STT psum+negd+q)
V_DMA = "act"  # "act" ring | "sp" ring
RDOT_DT = F32  # dots accum_out dtype


def build_bass(n_tiles=W_PER_CORE // P, repeats=1):
    """repeats>1 unrolls the whole tile loop N times (straight-line) — used
    for marginal exec-time measurement (no NTFF profiling in this image)."""
    nc = bass.Bass()
    q_d = nc.declare_dram_parameter("q", [n_tiles * P, D], BF16, isOutput=False)
    k_d = nc.declare_dram_parameter("k", [n_tiles * P, FACTOR, D], BF16, isOutput=False)
    v_d = nc.declare_dram_parameter("v", [n_tiles * P, FACTOR, D], BF16, isOutput=False)
    id_d = nc.declare_dram_parameter("ident", [P, P], BF16, isOutput=False)
    o_d = nc.declare_dram_parameter("out", [n_tiles * P, D], BF16, isOutput=True)

    lp = nc.allow_low_precision(reason="bf16 stats/accums: rel_err gate is 2e-2")
    lp.__enter__()

    with tile.TileContext(nc) as tc:
        with (
            tc.tile_pool(name="qp", bufs=3) as qp,
            tc.tile_pool(name="kp", bufs=3) as kp,
            tc.tile_pool(name="vp", bufs=3) as vp,
            tc.tile_pool(name="outp", bufs=3) as outp,
            tc.tile_pool(name="scratch", bufs=2) as scratch,
            tc.tile_pool(name="smalls", bufs=3) as sm,
            tc.tile_pool(name="const", bufs=1) as cp,
            tc.tile_pool(name="psum", bufs=3, space="PSUM") as pp,
        ):
            ident = cp.tile([P, P], BF16)
            nc.sync.dma_start(ident[:], id_d[:])

            for _rep in range(repeats):
                for t in range(n_tiles):
                    rows = slice(t * P, (t + 1) * P)
                    q_sb = qp.tile([P, D], BF16)
                    nc.sync.dma_start(q_sb[:], q_d[rows, :])
                    k_sb = kp.tile([P, FACTOR, D], BF16)
                    nc.sync.dma_start(k_sb[:], k_d[rows, :, :])
                    v_sb = vp.tile([P, FACTOR, D], BF16)
                    if V_DMA == "act":
                        nc.scalar.dma_start(v_sb[:], v_d[rows, :, :])
                    else:
                        nc.sync.dma_start(v_sb[:], v_d[rows, :, :])

                    # ---- q stats
                    if Q_MODE == "bn":
                        bnst_q = sm.tile([P, 2, 6], STAT_DT)
                        for ch in range(2):
                            nc.vector.bn_stats(
                                bnst_q[:, ch], q_sb[:, ch * HALF : (ch + 1) * HALF]
                            )
                        aggr_q = sm.tile([P, 2], F32)  # (mean, var)
                        nc.vector.bn_aggr(
                            aggr_q[:], bnst_q[:].rearrange("p c x -> p (c x)")
                        )
                        mu_q = aggr_q[:, 0:1]
                        var_q = aggr_q[:, 1:2]
                    else:  # act2: Sigma q, Sigma q^2 via ACT accum
                        sum_q = sm.tile([P, 1], F32)
                        ssq_q = sm.tile([P, 1], F32)
                        dmpq = scratch.tile([P, D], BF16, tag="actdump")
                        nc.scalar.activation(
                            dmpq[:], q_sb[:], ACTF.Copy, accum_out=sum_q[:]
                        )
                        dmpq2 = scratch.tile([P, D], BF16, tag="actdump")
                        nc.scalar.activation(
                            dmpq2[:], q_sb[:], ACTF.Square, accum_out=ssq_q[:]
                        )
                        mu_q_t = sm.tile([P, 1], F32)
                        nc.gpsimd.tensor_scalar_mul(mu_q_t[:], sum_q[:], 1.0 / D)
                        mmq = sm.tile([P, 1], F32)
                        nc.gpsimd.tensor_mul(mmq[:], mu_q_t[:], mu_q_t[:])
                        var_q_t = sm.tile([P, 1], F32)
                        nc.vector.scalar_tensor_tensor(
                            var_q_t[:], ssq_q[:], 1.0 / D, mmq[:], ALU.mult,
                            ALU.subtract,
                        )
                        mu_q = mu_q_t[:]
                        var_q = var_q_t[:]

                    # ---- k stats: bn_stats on DVE (fused mean+var, one pass)
                    bnst_k = sm.tile([P, FACTOR, 2, 6], STAT_DT)
                    aggr_k = sm.tile([P, FACTOR, 2], F32)
                    for f in range(FACTOR):
                        for ch in range(2):
                            nc.vector.bn_stats(
                                bnst_k[:, f, ch],
                                k_sb[:, f, ch * HALF : (ch + 1) * HALF],
                            )
                        nc.vector.bn_aggr(
                            aggr_k[:, f], bnst_k[:, f].rearrange("p c x -> p (c x)")
                        )

                    # ---- v stats
                    if V_MODE == "bn":
                        bnst_v = sm.tile([P, FACTOR, 2, 6], STAT_DT)
                        aggr_v = sm.tile([P, FACTOR, 2], F32)
                        for f in range(FACTOR):
                            for ch in range(2):
                                nc.vector.bn_stats(
                                    bnst_v[:, f, ch],
                                    v_sb[:, f, ch * HALF : (ch + 1) * HALF],
                                )
                            nc.vector.bn_aggr(
                                aggr_v[:, f],
                                bnst_v[:, f].rearrange("p c x -> p (c x)"),
                            )
                        mu_v = aggr_v[:, :, 0]
                        var_v = aggr_v[:, :, 1]
                    else:  # split: Sigma v^2 on ACT, Sigma v on DVE reduce
                        ssq_v = sm.tile([P, FACTOR], F32)
                        for f in range(FACTOR):
                            dmp = scratch.tile([P, D], BF16, tag="actdump")
                            nc.scalar.activation(
                                dmp[:], v_sb[:, f], ACTF.Square,
                                accum_out=ssq_v[:, f : f + 1],
                            )
                        sums_v = sm.tile([P, FACTOR], F32)
                        nc.vector.tensor_reduce(
                            sums_v[:], v_sb[:], AXL.X, ALU.add
                        )
                        mu_v_t = sm.tile([P, FACTOR], F32)
                        nc.gpsimd.tensor_scalar_mul(mu_v_t[:], sums_v[:], 1.0 / D)
                        mm_v = sm.tile([P, FACTOR], F32)
                        nc.gpsimd.tensor_mul(mm_v[:], mu_v_t[:], mu_v_t[:])
                        var_v_t = sm.tile([P, FACTOR], F32)
                        nc.vector.scalar_tensor_tensor(
                            var_v_t[:], ssq_v[:], 1.0 / D, mm_v[:], ALU.mult,
                            ALU.subtract,
                        )
                        mu_v = mu_v_t[:]
                        var_v = var_v_t[:]

                    # ---- rstd = 1/sqrt(var+eps): DVE recip + ACT sqrt
                    tq = sm.tile([P, 1], F32)
                    nc.gpsimd.tensor_scalar_add(tq[:], var_q, LN_EPS)
                    rq = sm.tile([P, 1], F32)
                    nc.vector.reciprocal(rq[:], tq[:])
                    aq = sm.tile([P, 1], F32)
                    nc.scalar.sqrt(aq[:], rq[:])

                    tk = sm.tile([P, FACTOR], F32)
                    nc.gpsimd.tensor_scalar_add(tk[:], aggr_k[:, :, 1], LN_EPS)
                    rk = sm.tile([P, FACTOR], F32)
                    nc.vector.reciprocal(rk[:], tk[:])
                    ak = sm.tile([P, FACTOR], F32)
                    nc.scalar.sqrt(ak[:], rk[:])

                    tv = sm.tile([P, FACTOR], F32)
                    nc.gpsimd.tensor_scalar_add(tv[:], var_v, LN_EPS)
                    rv = sm.tile([P, FACTOR], F32)
                    nc.vector.reciprocal(rv[:], tv[:])
                    av = sm.tile([P, FACTOR], F32)
                    nc.scalar.sqrt(av[:], rv[:])

                    # ---- raw dots r_f = q . k_f via DVE STT with accum
                    rdots = sm.tile([P, FACTOR], RDOT_DT)
                    for f in range(FACTOR):
                        prod = scratch.tile([P, D], BF16, tag="prod")
                        nc.vector.scalar_tensor_tensor(
                            prod[:],
                            q_sb[:],
                            0.0,
                            k_sb[:, f],
                            ALU.bypass,
                            ALU.mult,
                            accum_out=rdots[:, f : f + 1],
                        )

                    # ---- w_f = aq*ak_f*(r_f - D*muq*muk_f); c_f = w_f*av_f
                    # small [P,4] chain on Pool (tensor_scalar / tensor_tensor)
                    t1 = sm.tile([P, FACTOR], F32)
                    nc.gpsimd.tensor_scalar(
                        t1[:], aggr_k[:, :, 0], mu_q, None, ALU.mult
                    )
                    rd32 = rdots[:]
                    t2 = sm.tile([P, FACTOR], F32)
                    nc.vector.scalar_tensor_tensor(
                        t2[:], t1[:], -float(D), rd32, ALU.mult, ALU.add
                    )
                    u = sm.tile([P, FACTOR], F32)
                    nc.gpsimd.tensor_scalar(u[:], ak[:], aq[:, 0:1], None, ALU.mult)
                    w = sm.tile([P, FACTOR], F32)
                    nc.gpsimd.tensor_mul(w[:], t2[:], u[:])
                    c = sm.tile([P, FACTOR], F32)
                    nc.gpsimd.tensor_mul(c[:], w[:], av[:])
                    e = sm.tile([P, FACTOR], F32)
                    nc.gpsimd.tensor_mul(e[:], c[:], mu_v)
                    neg_d = sm.tile([P, 1], F32)
                    nc.vector.tensor_reduce(neg_d[:], e[:], AXL.X, ALU.add, negate=True)

                    # ---- diag(c_f) on Pool
                    diags = []
                    for f in range(FACTOR):
                        dg = sm.tile([P, P], BF16, tag=f"diag{f}")
                        nc.gpsimd.tensor_scalar_mul(dg[:], ident[:], c[:, f : f + 1])
                        diags.append(dg)

                    # ---- PE: psum[s,:] = q[s,:] + sum_f c_f[s]*v_f[s,:]
                    psum_t = pp.tile([P, 2, HALF], F32)
                    if OUT_MODE == "act":
                        for h in range(2):
                            nc.tensor.matmul(
                                psum_t[:, h],
                                ident[:],
                                q_sb[:, h * HALF : (h + 1) * HALF],
                                start=True,
                                stop=False,
                            )
                        first = False
                    else:
                        first = True
                    for f in range(FACTOR):
                        for h in range(2):
                            nc.tensor.matmul(
                                psum_t[:, h],
                                diags[f][:],
                                v_sb[:, f, h * HALF : (h + 1) * HALF],
                                start=first,
                                stop=(f == FACTOR - 1),
                            )
                        first = False

                    # ---- out
                    out_sb = outp.tile([P, D], BF16)
                    if OUT_MODE == "act":
                        nc.scalar.activation(
                            out_sb[:],
                            psum_t[:].rearrange("p c x -> p (c x)"),
                            ACTF.Identity,
                            bias=neg_d[:],
                        )
                    else:  # dve: out = (psum + neg_d) + q
                        nc.vector.scalar_tensor_tensor(
                            out_sb[:],
                            psum_t[:].rearrange("p c x -> p (c x)"),
                            neg_d[:],
                            q_sb[:],
                            ALU.add,
                            ALU.add,
                        )
                    nc.sync.dma_start(o_d[rows, :], out_sb[:])
    return nc


def make_in_map(q_core, k_core, v_core):
    """Host-side per-core input prep shared by run()/test/sim: cast to bf16."""
    import ml_dtypes

    bf = ml_dtypes.bfloat16
    return {
        "q": np.ascontiguousarray(np.asarray(q_core, dtype=np.float32)).astype(bf),
        "k": np.ascontiguousarray(np.asarray(k_core, dtype=np.float32)).astype(bf),
        "v": np.ascontiguousarray(np.asarray(v_core, dtype=np.float32)).astype(bf),
        "ident": np.eye(P, dtype=np.float32).astype(bf),
    }


_NC_CACHE = None


def _get_nc():
    global _NC_CACHE
    if _NC_CACHE is None:
        _NC_CACHE = build_bass()
    return _NC_CACHE


def _numpy_reference(query, key, value, ln_w, ln_b):
    def ln(x):
        mu = x.mean(-1, keepdims=True)
        var = ((x - mu) ** 2).mean(-1, keepdims=True)
        return (x - mu) / np.sqrt(var + LN_EPS) * ln_w + ln_b

    qn, kn, vn = ln(query), ln(key), ln(value)
    b, s, d = key.shape
    k_win = kn.reshape(b, s // FACTOR, FACTOR, d)
    wts = np.einsum("bsd,bsfd->bsf", qn, k_win).reshape(b, s)
    attn = (wts[:, :, None] * vn).reshape(b, s // FACTOR, FACTOR, d).sum(axis=2)
    return (query + attn).astype(np.float32)


def run(inputs, trace=False):
    """Returns (full_output, BassKernelResults-or-None)."""
    query = np.asarray(inputs["query"], dtype=np.float32)
    key = np.asarray(inputs["key"], dtype=np.float32)
    value = np.asarray(inputs["value"], dtype=np.float32)
    ln_w = np.asarray(inputs["ln_weight"], dtype=np.float32)
    ln_b = np.asarray(inputs["ln_bias"], dtype=np.float32)

    if not (np.all(ln_w == 1.0) and np.all(ln_b == 0.0)):
        # General-path fallback (setup_inputs always produces ones/zeros).
        return _numpy_reference(query, key, value, ln_w, ln_b), None

    sq_h = SQ // 2  # 1024 query rows per core
    skv_h = SKV // 2  # 4096 kv rows per core
    in_maps = []
    for cidx in range(N_CORES):
        bi, h = divmod(cidx, 2)
        in_maps.append(
            make_in_map(
                query[bi, h * sq_h : (h + 1) * sq_h],
                key[bi, h * skv_h : (h + 1) * skv_h].reshape(W_PER_CORE, FACTOR, D),
                value[bi, h * skv_h : (h + 1) * skv_h].reshape(W_PER_CORE, FACTOR, D),
            )
        )

    res = run_bass_kernel_spmd(
        _get_nc(), in_maps, core_ids=list(range(N_CORES)), trace=trace
    )
    out = np.empty((B, SQ, D), dtype=np.float32)
    for cidx in range(N_CORES):
        bi, h = divmod(cidx, 2)
        out[bi, h * sq_h : (h + 1) * sq_h] = np.asarray(
            res.results[cidx]["out"], dtype=np.float32
        )
    return out, res


def kernel(**inputs) -> np.ndarray:
    out, _ = run(inputs)
    return out
